# revision 18
# baseline (speedup 1.0000x reference)
"""Autoregressive LSTM (128 warmup steps + 47 autoregressive decode steps) on
8 Trainium2 NeuronCores.

Strategy (TP-8 over the 4U gate dimension):
 - Each core owns a 512-column slice of the gate matrices (its 128 units x
   4 gates, column order [i|f|o|g]) and computes z = [x_t; h] @ W_loc for the
   full batch of 128.
 - h is sharded by units across cores; each step the 8 hT tiles (128x128)
   are exchanged with ONE remote_dma_broadcast per core: every core
   broadcasts its transposed h tile to all 8 cores' SBUF (self included)
   into gather slot <own id> (an If-chain on partition_id selects among 8
   static broadcast variants).  Sender groups 0-3 / 4-7 use separate remote
   semaphores so the receiver can start consuming the first 4 slots before
   the rest arrive.
 - The decode dense feedback is folded into the recurrent weights host-side:
     z_{t+1} = Wx(Wd^T h + bd) + Wh h + b = (Wh + Wd@Wx) h + (b + bd@Wx)
   so decode steps need no per-step dense matmul; predictions are computed
   at the end from the stored h tiles in one batched matmul per core
   (partial over that core's units), summed on the host.
"""

import sys

sys.path.insert(0, "/opt/trn_rl_repo")

import numpy as np
from concourse import bass, bacc, mybir

F32 = mybir.dt.float32
BF16 = mybir.dt.bfloat16

B = 128          # batch
F = 128          # features
U = 1024         # LSTM units
G = 512          # gate columns per core (4 * U / 8)
NC = 8           # cores

_GRAPH_CACHE = {}


def build_graph(warm_t=128, out_steps=48, n_dummy=66, n_pre=18):
    key = (warm_t, out_steps, n_dummy, n_pre)
    if key in _GRAPH_CACHE:
        return _GRAPH_CACHE[key]

    n_steps = warm_t + out_steps - 1          # total LSTM cell steps
    store0 = warm_t - 1                       # first step whose h is stored
    n_store = out_steps                       # h tiles stored for the dense tail
    n_dense = (n_store * B) // 512            # 512-col chunks in dense tail

    nc = bacc.Bacc(None, target_bir_lowering=False)

    # ---- DRAM parameters (per-core inputs prepared by the host) ----
    xT_d = nc.declare_dram_parameter("xT", [F, warm_t * B], BF16, isOutput=False)
    Ww_d = nc.declare_dram_parameter("Ww", [128, 9 * G], BF16, isOutput=False)
    Wdec_d = nc.declare_dram_parameter("Wdec", [128, 8 * G], BF16, isOutput=False)
    Wd_d = nc.declare_dram_parameter("Wdd", [128, F], BF16, isOutput=False)
    bw_d = nc.declare_dram_parameter("bw", [1, G], BF16, isOutput=False)
    bdec_d = nc.declare_dram_parameter("bdec", [1, G], BF16, isOutput=False)
    ident_d = nc.declare_dram_parameter("ident", [128, 128], BF16, isOutput=False)
    ones_d = nc.declare_dram_parameter("ones", [1, 128], BF16, isOutput=False)
    out_d = nc.declare_dram_parameter("out", [F, n_store * B], F32, isOutput=True)

    # ---- SBUF ----
    xT_s = nc.alloc_sbuf_tensor("xT_s", [F, warm_t * B], BF16)
    Ww_s = nc.alloc_sbuf_tensor("Ww_s", [128, 9 * G], BF16)
    Wdec_s = nc.alloc_sbuf_tensor("Wdec_s", [128, 8 * G], BF16)
    Wd_s = nc.alloc_sbuf_tensor("Wd_s", [128, F], BF16)
    bw_s = nc.alloc_sbuf_tensor("bw_s", [1, G], BF16)
    bdec_s = nc.alloc_sbuf_tensor("bdec_s", [1, G], BF16)
    ident_s = nc.alloc_sbuf_tensor("ident_s", [128, 128], BF16)
    ones_s = nc.alloc_sbuf_tensor("ones_s", [1, 128], BF16)

    gath = [nc.alloc_sbuf_tensor(f"gath{p}", [128, NC, B], BF16) for p in (0, 1)]
    send_s = nc.alloc_sbuf_tensor("send_s", [128, 2, B], BF16)
    h_store = nc.alloc_sbuf_tensor("h_store", [128, n_store * B], BF16)
    sig_ifo = [nc.alloc_sbuf_tensor(f"sig_ifo{p}", [B, 384], F32) for p in (0, 1)]
    g_t = [nc.alloc_sbuf_tensor(f"g_t{p}", [B, 128], F32) for p in (0, 1)]
    tanh_c = [nc.alloc_sbuf_tensor(f"tanh_c{p}", [B, 128], F32) for p in (0, 1)]
    h_new = [nc.alloc_sbuf_tensor(f"h_new{p}", [B, 128], BF16) for p in (0, 1)]
    m1_s = nc.alloc_sbuf_tensor("m1_s", [B, 128], F32)
    m2_s = nc.alloc_sbuf_tensor("m2_s", [B, 128], F32)
    pT_s = nc.alloc_sbuf_tensor("pT_s", [F, n_store * B], F32)

    # ---- PSUM ----
    z_ps = [nc.alloc_psum_tensor(f"z_ps{p}", [B, G], F32) for p in (0, 1)]
    tr_ps = [nc.alloc_psum_tensor(f"tr_ps{p}", [128, B], BF16) for p in (0, 1)]
    pd_ps = [nc.alloc_psum_tensor(f"pd_ps{p}", [F, 512], F32) for p in (0, 1)]
    c_ps = nc.alloc_psum_tensor("c_ps", [B, 128], F32)
    dummy_ps = nc.alloc_psum_tensor("dummy_ps", [128, 128], F32)

    def src_ap(t):
        """SBUF source of the transposed h tile for step t (broadcast input)."""
        if t >= store0:
            return h_store.ap()[:, (t - store0) * B:(t - store0 + 1) * B]
        return send_s.ap()[:, t % 2, :]

    with (
        nc.Block() as block,
        nc.semaphore("dma_init") as dma_init,
        nc.semaphore("dma_init2") as dma_init2,
        nc.semaphore("init_sem") as init_sem,
        nc.semaphore("rsemA") as rsemA,   # senders 0-3, +8 per round
        nc.semaphore("rsemB") as rsemB,   # senders 4-7, +8 per round
        nc.semaphore("lsem") as lsem,     # local broadcast completion, +16/round
        nc.semaphore("prep_sem") as prep_sem,
        nc.semaphore("sem_z") as sem_z,
        nc.semaphore("sem_tr") as sem_tr,
        nc.semaphore("sem_act") as sem_act,
        nc.semaphore("sem_dve") as sem_dve,
        nc.semaphore("sem_vcp") as sem_vcp,
    ):
        @block.sync
        def _(sp):
            for dst, src in (
                (Ww_s, Ww_d), (bw_s, bw_d), (ident_s, ident_d), (ones_s, ones_d),
                (xT_s, xT_d),
            ):
                sp.dma_start(out=dst.ap(), in_=src[:]).then_inc(dma_init, 16)
            for dst, src in ((Wdec_s, Wdec_d), (bdec_s, bdec_d), (Wd_s, Wd_d)):
                sp.dma_start(out=dst.ap(), in_=src[:]).then_inc(dma_init2, 16)
            # final output DMA
            sp.wait_ge(sem_act, 4 * n_steps + n_dense)
            sp.dma_start(out=out_d[:], in_=pT_s.ap()).then_inc(dma_init2, 16)

        @block.gpsimd
        def _(g):
            g.bir_kernel_barrier_wait([list(range(NC))])
            pid = g.partition_id()
            rd = [(0, d) for d in range(8)]
            for t in range(n_steps - 1):
                g.wait_ge(sem_z, t + 1)  # keep Q7 descgen off the in-flight chain
                for r in range(NC):
                    with g.If_eq(pid, r):
                        g.remote_dma_broadcast(
                            out_ap=gath[(t + 1) % 2].ap()[:, r, :],
                            in_ap=src_ap(t),
                            remote_sem=rsemA if r < 4 else rsemB,
                            local_sem=lsem,
                            rdests=rd,
                        ).then_inc(prep_sem, 1)
                g.wait_ge(prep_sem, t + 1)
                g.wait_ge(sem_act, 4 * t + 4)  # scalar copy half staged
                g.wait_ge(sem_vcp, t + 1)      # vector copy half staged
                g.trigger_dma(count=1)

        @block.vector
        def _(v):
            v.memset(gath[0].ap(), 0.0)
            v.memset(c_ps.ap(), 0.0)
            v.drain().then_inc(init_sem, 1)
            for t in range(n_steps):
                p = t % 2
                v.wait_ge(sem_act, 4 * t + 1)
                v.tensor_mul(m1_s.ap(), sig_ifo[p].ap()[:, 128:256], c_ps.ap())
                v.wait_ge(sem_act, 4 * t + 2)
                v.tensor_mul(m2_s.ap(), sig_ifo[p].ap()[:, 0:128], g_t[p].ap())
                v.drain()
                v.tensor_add(c_ps.ap(), m1_s.ap(), m2_s.ap())
                v.drain().then_inc(sem_dve, 1)
                v.wait_ge(sem_act, 4 * t + 3)
                if t >= 2:
                    v.wait_ge(sem_tr, t - 1)  # WAR: transpose t-2 read h_new bank
                v.tensor_mul(h_new[p].ap(), sig_ifo[p].ap()[:, 256:384], tanh_c[p].ap())
                v.drain().then_inc(sem_dve, 1)
                v.wait_ge(sem_tr, t + 1)
                if 2 <= t < store0:
                    v.wait_ge(lsem, 16 * (t - 1))
                v.tensor_copy(src_ap(t)[:, 64:128], tr_ps[p].ap()[:, 64:128])
                v.drain().then_inc(sem_vcp, 1)

        @block.scalar
        def _(a):
            Sig = mybir.ActivationFunctionType.Sigmoid
            Tanh = mybir.ActivationFunctionType.Tanh
            Copy = mybir.ActivationFunctionType.Copy
            for t in range(n_steps):
                p = t % 2
                a.wait_ge(sem_z, t + 1)
                if t >= 2:
                    a.wait_ge(sem_dve, 2 * (t - 2) + 2)  # WAR on sig/g_t banks
                a.activation(sig_ifo[p].ap()[:, 0:256], z_ps[p].ap()[:, 0:256], Sig).then_inc(sem_act, 1)
                a.activation(g_t[p].ap(), z_ps[p].ap()[:, 384:512], Tanh).then_inc(sem_act, 1)
                a.activation(sig_ifo[p].ap()[:, 256:384], z_ps[p].ap()[:, 256:384], Sig)
                a.wait_ge(sem_dve, 2 * t + 1)
                a.activation(tanh_c[p].ap(), c_ps.ap(), Tanh).then_inc(sem_act, 1)
                # stage copy: transposed h tile PSUM -> SBUF (broadcast source)
                a.wait_ge(sem_tr, t + 1)
                if 2 <= t < store0:
                    a.wait_ge(lsem, 16 * (t - 1))  # WAR: send_s bank of t-2 sent
                a.activation(src_ap(t)[:, 0:64], tr_ps[p].ap()[:, 0:64], Copy).then_inc(sem_act, 1)
            # dense tail: copy PSUM chunks to SBUF
            for q in range(n_dense):
                a.wait_ge(sem_z, n_steps + q + 1)
                a.activation(pT_s.ap()[:, 512 * q:512 * (q + 1)], pd_ps[q % 2].ap(), Copy).then_inc(sem_act, 1)

        @block.tensor
        def _(pe):
            def bias_x(t):
                """Pre-issue bias (+ x for warmup) matmuls for step t."""
                p = t % 2
                warm = t < warm_t
                pe.matmul(
                    z_ps[p].ap(), ones_s.ap(),
                    (bw_s if warm else bdec_s).ap(),
                    start=True, stop=False,
                )
                if warm:
                    pe.matmul(
                        z_ps[p].ap(),
                        xT_s.ap()[:, t * B:(t + 1) * B],
                        Ww_s.ap()[:, 0:G],
                        start=False, stop=False,
                    )

            pe.wait_ge(init_sem, 1)
            pe.wait_ge(dma_init, 16 * 5)  # warmup-phase loads only
            bias_x(0)
            for t in range(n_steps):
                p = t % 2
                warm = t < warm_t
                W_s = Ww_s if warm else Wdec_s
                blk = 1 if warm else 0
                gt = gath[t % 2]
                pe.wait_ge(rsemA, 8 * t)
                for j in range(4):
                    pe.matmul(
                        z_ps[p].ap(), gt.ap()[:, j, :],
                        W_s.ap()[:, (blk + j) * G:(blk + j + 1) * G],
                        start=False, stop=False,
                    )
                pe.wait_ge(rsemB, 8 * t)
                for j in range(4, 8):
                    last = j == 7
                    mm = pe.matmul(
                        z_ps[p].ap(), gt.ap()[:, j, :],
                        W_s.ap()[:, (blk + j) * G:(blk + j + 1) * G],
                        start=False, stop=last,
                    )
                    if last:
                        mm.then_inc(sem_z, 1)
                # pre-issue next step's bias/x while activations run
                if t + 1 < n_steps:
                    if t + 1 == warm_t:
                        pe.wait_ge(dma_init2, 16 * 3)  # decode weights loaded
                    bias_x(t + 1)
                for _ in range(n_pre):
                    pe.matmul(
                        dummy_ps.ap(), Ww_s.ap()[:, 0:128], Ww_s.ap()[:, 0:128],
                        start=True, stop=True,
                    )
                # transpose h_new -> [u, b]
                pe.wait_ge(sem_dve, 2 * t + 2)
                if t >= 2:
                    pe.wait_ge(sem_act, 4 * (t - 2) + 4)  # WAR: stage copy t-2
                    pe.wait_ge(sem_vcp, t - 1)            # WAR: vector half t-2
                pe.transpose(tr_ps[p].ap(), h_new[p].ap(), ident_s.ap()).then_inc(sem_tr, 1)
                # keep the PE clock ungated through the exchange window
                for _ in range(n_dummy):
                    pe.matmul(
                        dummy_ps.ap(), Ww_s.ap()[:, 0:128], Ww_s.ap()[:, 0:128],
                        start=True, stop=True,
                    )
            # dense tail: pT_partial = Wd_r.T @ h_store
            pe.wait_ge(sem_act, 4 * n_steps)
            for q in range(n_dense):
                if q >= 2:
                    pe.wait_ge(sem_act, 4 * n_steps + q - 1)
                pe.matmul(
                    pd_ps[q % 2].ap(), Wd_s.ap(),
                    h_store.ap()[:, 512 * q:512 * (q + 1)],
                    start=True, stop=True,
                ).then_inc(sem_z, 1)

    nc.compile()
    meta = dict(warm_t=warm_t, out_steps=out_steps, n_steps=n_steps, store0=store0)
    _GRAPH_CACHE[key] = (nc, meta)
    return nc, meta


def make_in_maps(x, Wx, Wh, b, Wd, bd, warm_t=128, out_steps=48):
    """Host-side prep: fold decode dense into recurrent weights, shard by core."""
    x = np.asarray(x, np.float32)
    Wx = np.asarray(Wx, np.float32)
    Wh = np.asarray(Wh, np.float32)
    b = np.asarray(b, np.float32)
    Wd = np.asarray(Wd, np.float32)
    bd = np.asarray(bd, np.float32)

    Wtil = (Wh.astype(np.float64) + Wd.astype(np.float64) @ Wx.astype(np.float64)).astype(np.float32)
    btil = (b.astype(np.float64) + bd.astype(np.float64) @ Wx.astype(np.float64)).astype(np.float32)

    import ml_dtypes
    bf16 = ml_dtypes.bfloat16
    xT = np.ascontiguousarray(x.transpose(2, 1, 0)).reshape(F, warm_t * B).astype(bf16)
    ident = np.eye(128).astype(bf16)
    ones = np.ones((1, 128), bf16)

    in_maps = []
    for r in range(NC):
        # gate column order [i | f | o | g]; reference gate order is i,f,g,o
        col_idx = np.concatenate(
            [np.arange(128) + base + 128 * r for base in (0, 1024, 3072, 2048)]
        )
        Wwarm = np.concatenate([Wx, Wh], 0)[:, col_idx]          # [1152, 512]
        Ww_h = np.ascontiguousarray(
            Wwarm.reshape(9, 128, G).transpose(1, 0, 2)).reshape(128, 9 * G)
        Wdec_h = np.ascontiguousarray(
            Wtil[:, col_idx].reshape(8, 128, G).transpose(1, 0, 2)).reshape(128, 8 * G)
        in_maps.append({
            "xT": xT,
            "Ww": Ww_h.astype(bf16),
            "Wdec": Wdec_h.astype(bf16),
            "Wdd": np.ascontiguousarray(Wd[128 * r:128 * (r + 1), :]).astype(bf16),
            "bw": b[col_idx].reshape(1, G).astype(bf16),
            "bdec": btil[col_idx].reshape(1, G).astype(bf16),
            "ident": ident,
            "ones": ones,
        })
    return in_maps


def postprocess(results, bd, out_steps=48):
    """Sum per-core partial pT, add bias, reshape to [B, S, F]."""
    acc = np.zeros((F, out_steps * B), np.float64)
    for r in range(NC):
        acc += results[r]["out"].astype(np.float64)
    pT = acc.reshape(F, out_steps, B) + np.asarray(bd, np.float64)[:, None, None]
    return np.ascontiguousarray(pT.transpose(2, 1, 0)).astype(np.float32)


_LDW_PATCHED = False


def _patch_ldw_opt():
    global _LDW_PATCHED
    if _LDW_PATCHED:
        return
    from concourse import bass_utils as _bu
    _orig = _bu.run_command

    def _patched(cmd, **kw):
        cmd = [c.replace("--enable-ldw-opt=false", "--enable-ldw-opt=true")
               if isinstance(c, str) else c for c in cmd]
        return _orig(cmd, **kw)

    _bu.run_command = _patched
    _LDW_PATCHED = True


def kernel(x, Wx, Wh, b, Wd, bd):
    from concourse.bass_utils import run_bass_kernel_spmd
    _patch_ldw_opt()

    nc, _ = build_graph(128, 48)
    in_maps = make_in_maps(x, Wx, Wh, b, Wd, bd, 128, 48)
    res = run_bass_kernel_spmd(nc, in_maps, list(range(NC)))
    return postprocess(res.results, bd, 48)
